# revision 1
# baseline (speedup 1.0000x reference)
"""ChemConv Trainium2 kernel.

Computes, for A=2048 atoms, IN_DEPTH=D=128, OUT_DEPTH=O=128, FILTER_LEN=F=16:

  nc1[a,f,d]  = sum_b conn[a,b,f] * node[b,d]
  combined    = concat([nc1, bond], axis=2)            # (A, F, D+2)
  out[a,o]    = sum_{f,k} combined[a,f,k] * filters[o,f,k]

Sharding: atom rows of conn split across 8 NeuronCores (A/8 = 256 atoms each);
node/filters/bond replicated. No cross-core reduction.

Per-core kernel:
  Stage 1 contracts b = bo*16 + bi with bo on the 128 partitions and bi as 16
  PSUM-accumulated matmuls. This layout means every conn DMA descriptor moves
  1KB contiguous (16 b x 16 f x 4B), so the 32MB/core conn stream runs at full
  HBM rate with no transposes. Because the matmul RHS must be a single-stride
  AP, each (a,r) tile is reshuffled on-chip (DVE/ACT copies, overlapped under
  the DMA) into (bi, a, f) order before the matmuls. Matmuls run in fp32r
  (full PE rate at free-dim>=256; ~1.5e-4 rel err measured on HW).
  Stage 2 consumes nc1 held as [d, a, f] in SBUF: one matmul per f against
  PE-transposed filters, plus one K=32 matmul for the bond term, accumulating
  out_T[o, a] in PSUM. Host transposes/concats the per-core (128, 256) outputs.
"""

import numpy as np

import concourse.bacc as bacc
import concourse.mybir as mybir
import concourse.tile as tile
from concourse.bass_utils import run_bass_kernel_spmd

A, D, O, F = 2048, 128, 128, 16
NCORES = 8
AL = A // NCORES   # atoms per core = 256
NB = 16            # a-blocks per core
ABK = AL // NB     # atoms per block = 16
BO, BI = 128, 16   # b = bo*16 + bi

_f32 = mybir.dt.float32
_f32r = mybir.dt.float32r


def _build():
    nc = bacc.Bacc("TRN2", target_bir_lowering=False, debug=False)

    conn = nc.dram_tensor("conn", [AL, BO, BI * F], _f32r, kind="ExternalInput")
    node = nc.dram_tensor("node", [BO, BI, D], _f32r, kind="ExternalInput")
    filt = nc.dram_tensor("filt", [O, F, D + 2], _f32, kind="ExternalInput")
    bond = nc.dram_tensor("bond", [2, BO, F * 2], _f32, kind="ExternalInput")
    iden = nc.dram_tensor("iden", [128, 128], _f32, kind="ExternalInput")
    out = nc.dram_tensor("out", [O, AL], _f32, kind="ExternalOutput")

    with tile.TileContext(nc) as tc:
        with (
            tc.tile_pool(name="sb", bufs=1) as sb,
            tc.tile_pool(name="connp", bufs=4) as connp,
            tc.tile_pool(name="conn2p", bufs=3) as conn2p,
            tc.tile_pool(name="ps1", bufs=2, space="PSUM") as ps1,
            tc.tile_pool(name="pst", bufs=2, space="PSUM") as pst,
            tc.tile_pool(name="ps2", bufs=1, space="PSUM") as ps2,
        ):
            # First two conn blocks go ahead of everything on the sync ring so
            # the stream starts immediately; node follows them (needed only by
            # the first matmul), filt/bond ride the scalar ring.
            ct_pre = []
            for ab in range(2):
                ct = connp.tile([BO, ABK, BI * F], _f32r, tag="conn")
                eng = nc.sync if ab % 2 == 0 else nc.scalar
                eng.dma_start(
                    ct[:],
                    conn[ab * ABK : (ab + 1) * ABK].rearrange("a p r -> p a r"),
                )
                ct_pre.append(ct)
            node_sb = sb.tile([BO, BI, D], _f32r)
            nc.sync.dma_start(node_sb[:], node[:])
            filt_sb = sb.tile([O, F, D + 2], _f32)
            nc.scalar.dma_start(filt_sb[:], filt[:])
            bond_sb = sb.tile([BO, 2, F * 2], _f32)
            nc.scalar.dma_start(bond_sb[:], bond[:].rearrange("t p x -> p t x"))

            ident = sb.tile([128, 128], _f32)
            nc.scalar.dma_start(ident[:], iden[:])

            # filters[:, f, :D] -> filtT[d, f, o] via PE transpose (exact in f32)
            filtT_sb = sb.tile([D, F, O], _f32r)
            for f in range(F):
                pt = pst.tile([128, 128], _f32, tag="pt")
                nc.tensor.transpose(pt[:], filt_sb[:, f, 0:D], ident[:])
                nc.vector.tensor_copy(filtT_sb[:, f, :], pt[:])

            # filters[:, :, D:D+2] -> bfiltT[(f,j), o] : [32, 128]
            bf_tmp = sb.tile([O, F * 2], _f32)
            nc.vector.tensor_copy(
                bf_tmp[:].rearrange("o (f j) -> o f j", j=2),
                filt_sb[:, :, D : D + 2],
            )
            bfiltT_sb = sb.tile([F * 2, O], _f32r)
            ptb = pst.tile([F * 2, 128], _f32, tag="ptb")
            nc.tensor.transpose(ptb[:], bf_tmp[:], ident[:])
            nc.vector.tensor_copy(bfiltT_sb[:], ptb[:])

            # bond[a, (f,j)] -> bondT[(f,j), a] : [32, 256]
            bondT_sb = sb.tile([F * 2, AL], _f32r)
            for t in range(2):
                pb = pst.tile([F * 2, 128], _f32, tag="ptb")
                nc.tensor.transpose(pb[:], bond_sb[:, t, :], ident[:])
                nc.vector.tensor_copy(bondT_sb[:, t * 128 : (t + 1) * 128], pb[:])

            # Stage 1: nc1[d, a, f] = sum_b node[b, d] * conn[a, b, f]
            # Stage 2 runs per half (atoms 0:128 / 128:256) as soon as that
            # half's blocks are done, so only the second half sits in the tail.
            nc1_sb = sb.tile([D, AL, F], _f32r)
            out_sb = sb.tile([O, AL], _f32)
            HB = NB // 2  # blocks per half

            def stage2_half(h):
                a0 = h * (AL // 2)
                p2 = ps2.tile([O, AL // 2], _f32, tag="p2")
                for f in range(F):
                    nc.tensor.matmul(
                        p2[:],
                        filtT_sb[:, f, :],
                        nc1_sb[:, a0 : a0 + AL // 2, f],
                        start=(f == 0),
                        stop=False,
                    )
                nc.tensor.matmul(
                    p2[:],
                    bfiltT_sb[:],
                    bondT_sb[:, a0 : a0 + AL // 2],
                    start=False,
                    stop=True,
                )
                nc.vector.tensor_copy(out_sb[:, a0 : a0 + AL // 2], p2[:])
                nc.scalar.dma_start(out[:, a0 : a0 + AL // 2], out_sb[:, a0 : a0 + AL // 2])

            for ab in range(NB):
                if ab < 2:
                    ct = ct_pre[ab]
                else:
                    ct = connp.tile([BO, ABK, BI * F], _f32r, tag="conn")
                    eng = nc.sync if ab % 2 == 0 else nc.scalar
                    eng.dma_start(
                        ct[:],
                        conn[ab * ABK : (ab + 1) * ABK].rearrange("a p r -> p a r"),
                    )
                # reshuffle (a, bi, f) -> (bi, a, f) so each matmul RHS is one
                # contiguous slice (DVE/ACT, balanced by the scheduler)
                ct2 = conn2p.tile([BO, BI, ABK * F], _f32r, tag="conn2")
                for bi in range(BI):
                    nc.any.tensor_copy(
                        ct2[:, bi, :].rearrange("p (a f) -> p a f", f=F),
                        ct[:, :, bi * F : (bi + 1) * F],
                    )
                p1 = ps1.tile([D, ABK * F], _f32, tag="p1")
                for bi in range(BI):
                    nc.tensor.matmul(
                        p1[:],
                        node_sb[:, bi, :],
                        ct2[:, bi, :],
                        start=(bi == 0),
                        stop=(bi == BI - 1),
                    )
                nc.vector.tensor_copy(
                    nc1_sb[:, ab * ABK : (ab + 1) * ABK, :],
                    p1[:].rearrange("p (a f) -> p a f", f=F),
                )
                if ab == HB - 1:
                    stage2_half(0)
            stage2_half(1)

    nc.compile()
    return nc


def _in_maps(node_property_tensor, connectivity_tensor, bond_property_tensor, filters):
    node = np.asarray(node_property_tensor, dtype=np.float32).reshape(BO, BI, D)
    conn = np.asarray(connectivity_tensor, dtype=np.float32)
    bond = np.asarray(bond_property_tensor, dtype=np.float32)
    filt = np.asarray(filters, dtype=np.float32)
    maps = []
    for c in range(NCORES):
        maps.append(
            {
                "conn": conn[c * AL : (c + 1) * AL].reshape(AL, BO, BI * F),
                "node": node,
                "filt": filt,
                "bond": bond[c * AL : (c + 1) * AL].reshape(2, BO, F * 2),
                "iden": np.eye(128, dtype=np.float32),
            }
        )
    return maps


def _enable_tracing():
    """Install the NTFF profile hook (missing antenv.axon_hooks shim) and
    neuter the artifact upload (zero-egress container). Profiling only —
    never touched on the plain kernel() path."""
    import sys
    import types

    try:
        import antenv.axon_hooks  # noqa: F401
    except ImportError:
        from trn_agent_boot.trn_boot import _ntff_profile_via_ctypes

        hook = _ntff_profile_via_ctypes("/opt/axon/libaxon_pjrt.so")
        mod = types.ModuleType("antenv.axon_hooks")
        mod._hook = hook
        mod.get_axon_ntff_profile_hook = lambda: mod._hook
        mod.set_axon_ntff_profile_hook = lambda h: setattr(mod, "_hook", h)
        sys.modules["antenv.axon_hooks"] = mod
        import antenv

        antenv.axon_hooks = mod

    import concourse.bass_utils as _bu

    _bu.upload_artifacts = lambda tmpdir: tmpdir


def run(
    node_property_tensor,
    connectivity_tensor,
    bond_property_tensor,
    filters,
    trace=False,
):
    """Run the sharded kernel; returns (full (A, O) output, exec_time_ns|None)."""
    if trace:
        _enable_tracing()
    nc = _build()
    maps = _in_maps(
        node_property_tensor, connectivity_tensor, bond_property_tensor, filters
    )
    res = run_bass_kernel_spmd(nc, maps, core_ids=list(range(NCORES)), trace=trace)
    parts = [res.results[c]["out"] for c in range(NCORES)]  # each (O, AL)
    full = np.concatenate(parts, axis=1).T  # (A, O)
    return np.ascontiguousarray(full, dtype=np.float32), res.exec_time_ns


def kernel(
    node_property_tensor, connectivity_tensor, bond_property_tensor, filters
) -> np.ndarray:
    out, _ = run(
        node_property_tensor, connectivity_tensor, bond_property_tensor, filters
    )
    return out



# revision 2
# speedup vs baseline: 1.6990x; 1.6990x over previous
"""ChemConv Trainium2 kernel.

Computes, for A=2048 atoms, IN_DEPTH=D=128, OUT_DEPTH=O=128, FILTER_LEN=F=16:

  nc1[a,f,d]  = sum_b conn[a,b,f] * node[b,d]
  combined    = concat([nc1, bond], axis=2)            # (A, F, D+2)
  out[a,o]    = sum_{f,k} combined[a,f,k] * filters[o,f,k]

Sharding: atom rows of conn split across 8 NeuronCores (A/8 = 256 atoms each);
node/filters/bond replicated. No cross-core reduction.

The kernel is HBM-bound on the conn stream, so conn ships as bf16 (16.8MB/core
instead of 33.6MB; the b-contraction accumulates in fp32 PSUM, measured rel err
~2e-3 vs the 2e-2 gate). The host pre-packs conn into the exact SBUF layout the
matmuls consume -- per macro-block of 32 atoms: [bo=128 partitions][bi][a][f]
with b = bo*16 + bi -- so every DMA moves 16KB fully-contiguous per partition
and no on-chip reshuffle is needed. Filters/bond are host-pretransposed
(tiny), eliminating the PE transposes and the identity matrix of the fp32
version.

Per-core kernel:
  Stage 1 contracts b with bo on the 128 partitions and bi as 16
  PSUM-accumulated bf16 matmuls of free dim 512 (32 atoms x 16 f) per
  macro-block; PSUM (fp32) is copied to nc1[d, a, f] in SBUF as bf16.
  Stage 2 runs per half (128 atoms): one matmul per f against host-transposed
  filtT[d, f, o], plus one K=32 matmul for the bond term, accumulating
  out_T[o, a] in PSUM. Host transposes/concats the per-core (128, 256) outputs.
"""

import ml_dtypes
import numpy as np

import concourse.bacc as bacc
import concourse.mybir as mybir
import concourse.tile as tile
from concourse.bass_utils import run_bass_kernel_spmd

A, D, O, F = 2048, 128, 128, 16
NCORES = 8
AL = A // NCORES   # atoms per core = 256
MB = 8             # macro-blocks per core
ABK = AL // MB     # atoms per macro-block = 32
BO, BI = 128, 16   # b = bo*16 + bi

_f32 = mybir.dt.float32
_bf16 = mybir.dt.bfloat16
_np_bf16 = ml_dtypes.bfloat16


def _build():
    nc = bacc.Bacc("TRN2", target_bir_lowering=False, debug=False)

    conn = nc.dram_tensor("conn", [MB * BO, BI, ABK * F], _bf16, kind="ExternalInput")
    node = nc.dram_tensor("node", [BO, BI * D], _bf16, kind="ExternalInput")
    filtT = nc.dram_tensor("filtT", [D, F * O], _bf16, kind="ExternalInput")
    bfiltT = nc.dram_tensor("bfiltT", [F * 2, O], _bf16, kind="ExternalInput")
    bondT = nc.dram_tensor("bondT", [F * 2, AL], _bf16, kind="ExternalInput")
    out = nc.dram_tensor("out", [O, AL], _f32, kind="ExternalOutput")

    with tile.TileContext(nc) as tc:
        with (
            tc.tile_pool(name="sb", bufs=1) as sb,
            tc.tile_pool(name="connp", bufs=4) as connp,
            tc.tile_pool(name="ps1", bufs=2, space="PSUM") as ps1,
            tc.tile_pool(name="ps2", bufs=1, space="PSUM") as ps2,
        ):
            # First two conn blocks go ahead of everything on their rings so
            # the stream starts immediately; node (needed by the first matmul)
            # follows on the sync ring, filtT/bond ride the scalar ring.
            ct_pre = []
            for mb in range(2):
                ct = connp.tile([BO, BI, ABK * F], _bf16, tag="conn")
                eng = nc.sync if mb % 2 == 0 else nc.scalar
                eng.dma_start(ct[:], conn[mb * BO : (mb + 1) * BO])
                ct_pre.append(ct)
            node_sb = sb.tile([BO, BI * D], _bf16)
            nc.sync.dma_start(node_sb[:], node[:])
            filtT_sb = sb.tile([D, F * O], _bf16)
            nc.scalar.dma_start(filtT_sb[:], filtT[:])
            bfiltT_sb = sb.tile([F * 2, O], _bf16)
            nc.scalar.dma_start(bfiltT_sb[:], bfiltT[:])
            bondT_sb = sb.tile([F * 2, AL], _bf16)
            nc.scalar.dma_start(bondT_sb[:], bondT[:])

            # Stage 1: nc1[d, a, f] = sum_b node[b, d] * conn[a, b, f]
            # Stage 2 runs per half (atoms 0:128 / 128:256) as soon as that
            # half's blocks are done, so only the second half sits in the tail.
            nc1_sb = sb.tile([D, AL, F], _bf16)
            out_sb = sb.tile([O, AL], _f32)

            def stage2_half(h):
                a0 = h * (AL // 2)
                p2 = ps2.tile([O, AL // 2], _f32, tag="p2")
                for f in range(F):
                    nc.tensor.matmul(
                        p2[:],
                        filtT_sb[:, f * O : (f + 1) * O],
                        nc1_sb[:, a0 : a0 + AL // 2, f],
                        start=(f == 0),
                        stop=False,
                    )
                nc.tensor.matmul(
                    p2[:],
                    bfiltT_sb[:],
                    bondT_sb[:, a0 : a0 + AL // 2],
                    start=False,
                    stop=True,
                )
                nc.vector.tensor_copy(out_sb[:, a0 : a0 + AL // 2], p2[:])
                nc.scalar.dma_start(out[:, a0 : a0 + AL // 2], out_sb[:, a0 : a0 + AL // 2])

            for mb in range(MB):
                if mb < 2:
                    ct = ct_pre[mb]
                else:
                    ct = connp.tile([BO, BI, ABK * F], _bf16, tag="conn")
                    eng = nc.sync if mb % 2 == 0 else nc.scalar
                    eng.dma_start(ct[:], conn[mb * BO : (mb + 1) * BO])
                p1 = ps1.tile([D, ABK * F], _f32, tag="p1")
                for bi in range(BI):
                    nc.tensor.matmul(
                        p1[:],
                        node_sb[:, bi * D : (bi + 1) * D],
                        ct[:, bi, :],
                        start=(bi == 0),
                        stop=(bi == BI - 1),
                    )
                nc.any.tensor_copy(
                    nc1_sb[:, mb * ABK : (mb + 1) * ABK, :],
                    p1[:].rearrange("p (a f) -> p a f", f=F),
                )
                if mb == MB // 2 - 1:
                    stage2_half(0)
            stage2_half(1)

    nc.compile()
    return nc


def _in_maps(node_property_tensor, connectivity_tensor, bond_property_tensor, filters):
    node = np.asarray(node_property_tensor, dtype=np.float32)
    conn = np.asarray(connectivity_tensor, dtype=np.float32)
    bond = np.asarray(bond_property_tensor, dtype=np.float32)
    filt = np.asarray(filters, dtype=np.float32)

    node_p = np.ascontiguousarray(node.reshape(BO, BI * D)).astype(_np_bf16)
    # filters[o, f, :D] -> filtT[d, (f o)]
    filtT = np.ascontiguousarray(filt[:, :, :D].transpose(2, 1, 0)).astype(
        _np_bf16
    ).reshape(D, F * O)
    # filters[o, f, D:D+2] -> bfiltT[(f j), o]
    bfiltT = np.ascontiguousarray(filt[:, :, D:].transpose(1, 2, 0)).astype(
        _np_bf16
    ).reshape(F * 2, O)

    conn_bf = conn.astype(_np_bf16)
    maps = []
    for c in range(NCORES):
        cs = conn_bf[c * AL : (c + 1) * AL]  # (AL, B=2048, F)
        cp = np.ascontiguousarray(
            cs.reshape(MB, ABK, BO, BI, F).transpose(0, 2, 3, 1, 4)
        ).reshape(MB * BO, BI, ABK * F)
        bs = bond[c * AL : (c + 1) * AL]  # (AL, F, 2)
        bT = np.ascontiguousarray(bs.transpose(1, 2, 0)).astype(_np_bf16).reshape(
            F * 2, AL
        )
        maps.append(
            {
                "conn": cp,
                "node": node_p,
                "filtT": filtT,
                "bfiltT": bfiltT,
                "bondT": bT,
            }
        )
    return maps


def _enable_tracing():
    """Install the NTFF profile hook (missing antenv.axon_hooks shim) and
    neuter the artifact upload (zero-egress container). Profiling only —
    never touched on the plain kernel() path."""
    import sys
    import types

    try:
        import antenv.axon_hooks  # noqa: F401
    except ImportError:
        from trn_agent_boot.trn_boot import _ntff_profile_via_ctypes

        hook = _ntff_profile_via_ctypes("/opt/axon/libaxon_pjrt.so")
        mod = types.ModuleType("antenv.axon_hooks")
        mod._hook = hook
        mod.get_axon_ntff_profile_hook = lambda: mod._hook
        mod.set_axon_ntff_profile_hook = lambda h: setattr(mod, "_hook", h)
        sys.modules["antenv.axon_hooks"] = mod
        import antenv

        antenv.axon_hooks = mod

    import concourse.bass_utils as _bu

    _bu.upload_artifacts = lambda tmpdir: tmpdir


def run(
    node_property_tensor,
    connectivity_tensor,
    bond_property_tensor,
    filters,
    trace=False,
):
    """Run the sharded kernel; returns (full (A, O) output, exec_time_ns|None)."""
    if trace:
        _enable_tracing()
    nc = _build()
    maps = _in_maps(
        node_property_tensor, connectivity_tensor, bond_property_tensor, filters
    )
    res = run_bass_kernel_spmd(nc, maps, core_ids=list(range(NCORES)), trace=trace)
    parts = [res.results[c]["out"] for c in range(NCORES)]  # each (O, AL)
    full = np.concatenate(parts, axis=1).T  # (A, O)
    return np.ascontiguousarray(full, dtype=np.float32), res.exec_time_ns


def kernel(
    node_property_tensor, connectivity_tensor, bond_property_tensor, filters
) -> np.ndarray:
    out, _ = run(
        node_property_tensor, connectivity_tensor, bond_property_tensor, filters
    )
    return out


# revision 5
# speedup vs baseline: 1.9379x; 1.1406x over previous
"""ChemConv Trainium2 kernel.

Computes, for A=2048 atoms, IN_DEPTH=D=128, OUT_DEPTH=O=128, FILTER_LEN=F=16:

  nc1[a,f,d]  = sum_b conn[a,b,f] * node[b,d]
  combined    = concat([nc1, bond], axis=2)            # (A, F, D+2)
  out[a,o]    = sum_{f,k} combined[a,f,k] * filters[o,f,k]

Sharding: atom rows of conn split across 8 NeuronCores (A/8 = 256 atoms each);
node/filters/bond replicated. No cross-core reduction.

The kernel is HBM-bound on the conn stream, so conn ships as bf16 (16.8MB/core
instead of 33.6MB; the b-contraction accumulates in fp32 PSUM, measured rel err
~2e-3 vs the 2e-2 gate). The host pre-packs conn into the exact SBUF layout the
matmuls consume -- per macro-block of 32 atoms: [bo=128 partitions][bi][a][f]
with b = bo*16 + bi -- so every DMA moves 16KB fully-contiguous per partition
and no on-chip reshuffle is needed. Filters/bond are host-pretransposed
(tiny), eliminating the PE transposes and the identity matrix of the fp32
version.

Per-core kernel:
  Stage 1 contracts b with bo on the 128 partitions and bi as 16
  PSUM-accumulated bf16 matmuls of free dim 512 (32 atoms x 16 f) per
  macro-block; PSUM (fp32) is copied to nc1[d, a, f] in SBUF as bf16.
  Stage 2 runs per half (128 atoms): one matmul per f against host-transposed
  filtT[d, f, o], plus one K=32 matmul for the bond term, accumulating
  out_T[o, a] in PSUM. Host transposes/concats the per-core (128, 256) outputs.
"""

import ml_dtypes
import numpy as np

import concourse.bacc as bacc
import concourse.mybir as mybir
import concourse.tile as tile
from concourse.bass_utils import run_bass_kernel_spmd

A, D, O, F = 2048, 128, 128, 16
NCORES = 8
AL = A // NCORES   # atoms per core = 256
MB = 16            # macro-blocks per core
ABK = AL // MB     # atoms per macro-block = 16
BO, BI = 128, 16   # b = bo*16 + bi

_f32 = mybir.dt.float32
_bf16 = mybir.dt.bfloat16
_np_bf16 = ml_dtypes.bfloat16


def _build():
    nc = bacc.Bacc("TRN2", target_bir_lowering=False, debug=False)

    conn = nc.dram_tensor("conn", [MB * BO, BI, ABK * F], _bf16, kind="ExternalInput")
    node = nc.dram_tensor("node", [BO, BI * D], _bf16, kind="ExternalInput")
    filtT = nc.dram_tensor("filtT", [D, F * O], _bf16, kind="ExternalInput")
    bfiltT = nc.dram_tensor("bfiltT", [F * 2, O], _bf16, kind="ExternalInput")
    bondT = nc.dram_tensor("bondT", [F * 2, AL], _bf16, kind="ExternalInput")
    out = nc.dram_tensor("out", [O, AL], _f32, kind="ExternalOutput")

    with tile.TileContext(nc) as tc:
        with (
            tc.tile_pool(name="sb", bufs=1) as sb,
            tc.tile_pool(name="connp", bufs=4) as connp,
            tc.tile_pool(name="ps1", bufs=2, space="PSUM") as ps1,
            tc.tile_pool(name="ps2", bufs=1, space="PSUM") as ps2,
        ):
            # First two conn blocks go ahead of everything on their rings so
            # the stream starts immediately; node (needed by the first matmul)
            # follows on the sync ring, filtT/bond ride the scalar ring.
            ct_pre = []
            for mb in range(2):
                ct = connp.tile([BO, BI, ABK * F], _bf16, tag="conn")
                eng = nc.sync if mb % 2 == 0 else nc.scalar
                eng.dma_start(ct[:], conn[mb * BO : (mb + 1) * BO])
                ct_pre.append(ct)
            node_sb = sb.tile([BO, BI * D], _bf16)
            nc.sync.dma_start(node_sb[:], node[:])
            filtT_sb = sb.tile([D, F * O], _bf16)
            nc.scalar.dma_start(filtT_sb[:], filtT[:])
            bfiltT_sb = sb.tile([F * 2, O], _bf16)
            nc.scalar.dma_start(bfiltT_sb[:], bfiltT[:])
            bondT_sb = sb.tile([F * 2, AL], _bf16)
            nc.scalar.dma_start(bondT_sb[:], bondT[:])

            # Stage 1: nc1[d, f, a] = sum_b node[b, d] * conn[a, b, f]
            # (f-major so stage-2 rhs slices are contiguous). Stage 2 runs per
            # half (atoms 0:128 / 128:256) as soon as that half's blocks are
            # done, so only the second half sits in the tail.
            nc1_sb = sb.tile([D, F, AL], _bf16)
            out_sb = sb.tile([O, AL], _f32)

            def stage2_half(h):
                a0 = h * (AL // 2)
                p2 = ps2.tile([O, AL // 2], _f32, tag="p2")
                for f in range(F):
                    nc.tensor.matmul(
                        p2[:],
                        filtT_sb[:, f * O : (f + 1) * O],
                        nc1_sb[:, f, a0 : a0 + AL // 2],
                        start=(f == 0),
                        stop=False,
                    )
                nc.tensor.matmul(
                    p2[:],
                    bfiltT_sb[:],
                    bondT_sb[:, a0 : a0 + AL // 2],
                    start=False,
                    stop=True,
                )
                nc.vector.tensor_copy(out_sb[:, a0 : a0 + AL // 2], p2[:])
                nc.scalar.dma_start(out[:, a0 : a0 + AL // 2], out_sb[:, a0 : a0 + AL // 2])

            for mb in range(MB):
                if mb < 2:
                    ct = ct_pre[mb]
                else:
                    ct = connp.tile([BO, BI, ABK * F], _bf16, tag="conn")
                    eng = nc.sync if mb % 2 == 0 else nc.scalar
                    eng.dma_start(ct[:], conn[mb * BO : (mb + 1) * BO])
                p1 = ps1.tile([D, ABK * F], _f32, tag="p1")
                for bi in range(BI):
                    nc.tensor.matmul(
                        p1[:],
                        node_sb[:, bi * D : (bi + 1) * D],
                        ct[:, bi, :],
                        start=(bi == 0),
                        stop=(bi == BI - 1),
                    )
                nc.vector.tensor_copy(
                    nc1_sb[:, :, mb * ABK : (mb + 1) * ABK],
                    p1[:].rearrange("p (f a) -> p f a", a=ABK),
                )
                if mb == MB // 2 - 1:
                    stage2_half(0)
            stage2_half(1)

    nc.compile()
    return nc


def _in_maps(node_property_tensor, connectivity_tensor, bond_property_tensor, filters):
    node = np.asarray(node_property_tensor, dtype=np.float32)
    conn = np.asarray(connectivity_tensor, dtype=np.float32)
    bond = np.asarray(bond_property_tensor, dtype=np.float32)
    filt = np.asarray(filters, dtype=np.float32)

    node_p = np.ascontiguousarray(node.reshape(BO, BI * D)).astype(_np_bf16)
    # filters[o, f, :D] -> filtT[d, (f o)]
    filtT = np.ascontiguousarray(filt[:, :, :D].transpose(2, 1, 0)).astype(
        _np_bf16
    ).reshape(D, F * O)
    # filters[o, f, D:D+2] -> bfiltT[(f j), o]
    bfiltT = np.ascontiguousarray(filt[:, :, D:].transpose(1, 2, 0)).astype(
        _np_bf16
    ).reshape(F * 2, O)

    conn_bf = conn.astype(_np_bf16)
    maps = []
    for c in range(NCORES):
        cs = conn_bf[c * AL : (c + 1) * AL]  # (AL, B=2048, F)
        # pack [mb, bo, bi, f, a]: f-major per bi so stage-1 PSUM columns come
        # out (f, a) and stage-2 rhs slices are contiguous
        cp = np.ascontiguousarray(
            cs.reshape(MB, ABK, BO, BI, F).transpose(0, 2, 3, 4, 1)
        ).reshape(MB * BO, BI, ABK * F)
        bs = bond[c * AL : (c + 1) * AL]  # (AL, F, 2)
        bT = np.ascontiguousarray(bs.transpose(1, 2, 0)).astype(_np_bf16).reshape(
            F * 2, AL
        )
        maps.append(
            {
                "conn": cp,
                "node": node_p,
                "filtT": filtT,
                "bfiltT": bfiltT,
                "bondT": bT,
            }
        )
    return maps


def _enable_tracing():
    """Install the NTFF profile hook (missing antenv.axon_hooks shim) and
    neuter the artifact upload (zero-egress container). Profiling only —
    never touched on the plain kernel() path."""
    import sys
    import types

    try:
        import antenv.axon_hooks  # noqa: F401
    except ImportError:
        from trn_agent_boot.trn_boot import _ntff_profile_via_ctypes

        hook = _ntff_profile_via_ctypes("/opt/axon/libaxon_pjrt.so")
        mod = types.ModuleType("antenv.axon_hooks")
        mod._hook = hook
        mod.get_axon_ntff_profile_hook = lambda: mod._hook
        mod.set_axon_ntff_profile_hook = lambda h: setattr(mod, "_hook", h)
        sys.modules["antenv.axon_hooks"] = mod
        import antenv

        antenv.axon_hooks = mod

    import concourse.bass_utils as _bu

    _bu.upload_artifacts = lambda tmpdir: tmpdir


def run(
    node_property_tensor,
    connectivity_tensor,
    bond_property_tensor,
    filters,
    trace=False,
):
    """Run the sharded kernel; returns (full (A, O) output, exec_time_ns|None)."""
    if trace:
        _enable_tracing()
    nc = _build()
    maps = _in_maps(
        node_property_tensor, connectivity_tensor, bond_property_tensor, filters
    )
    res = run_bass_kernel_spmd(nc, maps, core_ids=list(range(NCORES)), trace=trace)
    parts = [res.results[c]["out"] for c in range(NCORES)]  # each (O, AL)
    full = np.concatenate(parts, axis=1).T  # (A, O)
    return np.ascontiguousarray(full, dtype=np.float32), res.exec_time_ns


def kernel(
    node_property_tensor, connectivity_tensor, bond_property_tensor, filters
) -> np.ndarray:
    out, _ = run(
        node_property_tensor, connectivity_tensor, bond_property_tensor, filters
    )
    return out


# revision 10
# speedup vs baseline: 2.2487x; 1.1604x over previous
"""ChemConv Trainium2 kernel.

Computes, for A=2048 atoms, IN_DEPTH=D=128, OUT_DEPTH=O=128, FILTER_LEN=F=16:

  nc1[a,f,d]  = sum_b conn[a,b,f] * node[b,d]
  combined    = concat([nc1, bond], axis=2)            # (A, F, D+2)
  out[a,o]    = sum_{f,k} combined[a,f,k] * filters[o,f,k]

Sharding: atom rows of conn split across 8 NeuronCores (A/8 = 256 atoms each);
node/filters/bond replicated. No cross-core reduction.

The kernel is HBM-bound on the conn stream, so conn ships as bf16 (16.8MB/core
instead of 33.6MB; the b-contraction accumulates in fp32 PSUM, measured rel err
~2e-3 vs the 2e-2 gate). The host pre-packs conn into the exact SBUF layout the
matmuls consume -- per macro-block of 32 atoms: [bo=128 partitions][bi][a][f]
with b = bo*16 + bi -- so every DMA moves 16KB fully-contiguous per partition
and no on-chip reshuffle is needed. Filters/bond are host-pretransposed
(tiny), eliminating the PE transposes and the identity matrix of the fp32
version.

Per-core kernel:
  Stage 1 contracts b with bo on the 128 partitions and bi as 16
  PSUM-accumulated bf16 matmuls of free dim 512 (32 atoms x 16 f) per
  macro-block; PSUM (fp32) is copied to nc1[d, a, f] in SBUF as bf16.
  Stage 2 runs per half (128 atoms): one matmul per f against host-transposed
  filtT[d, f, o], plus one K=32 matmul for the bond term, accumulating
  out_T[o, a] in PSUM. Host transposes/concats the per-core (128, 256) outputs.
"""

import ml_dtypes
import numpy as np

import concourse.bacc as bacc
import concourse.mybir as mybir
import concourse.tile as tile
from concourse.bass_utils import run_bass_kernel_spmd

A, D, O, F = 2048, 128, 128, 16
NCORES = 8
AL = A // NCORES   # atoms per core = 256
MB = 16            # macro-blocks per core
ABK = AL // MB     # atoms per macro-block = 16
BO, BI = 128, 16   # b = bo*16 + bi

_f32 = mybir.dt.float32
_bf16 = mybir.dt.bfloat16
_f8 = mybir.dt.float8e3
_np_bf16 = ml_dtypes.bfloat16
_np_f8 = ml_dtypes.float8_e3m4


def _build():
    nc = bacc.Bacc("TRN2", target_bir_lowering=False, debug=False)

    conn = nc.dram_tensor("conn", [MB * BO, BI, ABK * F], _f8, kind="ExternalInput")
    node = nc.dram_tensor("node", [BO, BI * D], _bf16, kind="ExternalInput")
    filtT = nc.dram_tensor("filtT", [D, F * O], _bf16, kind="ExternalInput")
    bfiltT = nc.dram_tensor("bfiltT", [F * 2, O], _bf16, kind="ExternalInput")
    bondT = nc.dram_tensor("bondT", [F * 2, AL], _bf16, kind="ExternalInput")
    out = nc.dram_tensor("out", [O, AL], _f32, kind="ExternalOutput")

    with tile.TileContext(nc) as tc:
        with (
            tc.tile_pool(name="sb", bufs=1) as sb,
            tc.tile_pool(name="connp", bufs=4) as connp,
            tc.tile_pool(name="ps1", bufs=2, space="PSUM") as ps1,
            tc.tile_pool(name="ps2", bufs=1, space="PSUM") as ps2,
        ):
            # node rides first on the sync ring (every stage-1 matmul needs
            # it), the first conn blocks follow; filtT/bond ride the scalar
            # ring behind ct1.
            node_sb = sb.tile([BO, BI * D], _bf16)
            nc.sync.dma_start(node_sb[:], node[:])
            ct_pre = []
            for mb in range(2):
                ct = connp.tile([BO, BI, ABK * F], _f8, tag="conn")
                eng = nc.scalar if mb % 2 == 0 else nc.sync
                eng.dma_start(ct[:], conn[mb * BO : (mb + 1) * BO])
                ct_pre.append(ct)
            filtT_sb = sb.tile([D, F * O], _bf16)
            nc.scalar.dma_start(filtT_sb[:], filtT[:])
            bfiltT_sb = sb.tile([F * 2, O], _bf16)
            nc.scalar.dma_start(bfiltT_sb[:], bfiltT[:])
            bondT_sb = sb.tile([F * 2, AL], _bf16)
            nc.scalar.dma_start(bondT_sb[:], bondT[:])

            # Stage 1: nc1[d, f, a] = sum_b node[b, d] * conn[a, b, f]
            # (f-major so stage-2 rhs slices are contiguous). Stage 2 runs per
            # half (atoms 0:128 / 128:256) as soon as that half's blocks are
            # done, so only the second half sits in the tail.
            nc1_sb = sb.tile([D, F, AL], _bf16)
            out_sb = sb.tile([O, AL], _f32)

            def stage2_half(h):
                a0 = h * (AL // 2)
                p2 = ps2.tile([O, AL // 2], _f32, tag="p2")
                for f in range(F):
                    nc.tensor.matmul(
                        p2[:],
                        filtT_sb[:, f * O : (f + 1) * O],
                        nc1_sb[:, f, a0 : a0 + AL // 2],
                        start=(f == 0),
                        stop=False,
                    )
                nc.tensor.matmul(
                    p2[:],
                    bfiltT_sb[:],
                    bondT_sb[:, a0 : a0 + AL // 2],
                    start=False,
                    stop=True,
                )
                nc.vector.tensor_copy(out_sb[:, a0 : a0 + AL // 2], p2[:])
                nc.scalar.dma_start(out[:, a0 : a0 + AL // 2], out_sb[:, a0 : a0 + AL // 2])

            for mb in range(MB):
                if mb < 2:
                    ct = ct_pre[mb]
                else:
                    ct = connp.tile([BO, BI, ABK * F], _f8, tag="conn")
                    eng = nc.scalar if mb % 2 == 0 else nc.sync
                    eng.dma_start(ct[:], conn[mb * BO : (mb + 1) * BO])
                p1 = ps1.tile([D, ABK * F], _f32, tag="p1")
                for bi in range(BI):
                    nc.tensor.matmul(
                        p1[:],
                        node_sb[:, bi * D : (bi + 1) * D],
                        ct[:, bi, :],
                        start=(bi == 0),
                        stop=(bi == BI - 1),
                    )
                nc.vector.tensor_copy(
                    nc1_sb[:, :, mb * ABK : (mb + 1) * ABK],
                    p1[:].rearrange("p (f a) -> p f a", a=ABK),
                )
                if mb == MB // 2 - 1:
                    stage2_half(0)
            stage2_half(1)

    nc.compile()
    return nc


def _in_maps(node_property_tensor, connectivity_tensor, bond_property_tensor, filters):
    node = np.asarray(node_property_tensor, dtype=np.float32)
    conn = np.asarray(connectivity_tensor, dtype=np.float32)
    bond = np.asarray(bond_property_tensor, dtype=np.float32)
    filt = np.asarray(filters, dtype=np.float32)

    node_p = np.ascontiguousarray(node.reshape(BO, BI * D)).astype(_np_bf16)
    # filters[o, f, :D] -> filtT[d, (f o)]
    filtT = np.ascontiguousarray(filt[:, :, :D].transpose(2, 1, 0)).astype(
        _np_bf16
    ).reshape(D, F * O)
    # filters[o, f, D:D+2] -> bfiltT[(f j), o]
    bfiltT = np.ascontiguousarray(filt[:, :, D:].transpose(1, 2, 0)).astype(
        _np_bf16
    ).reshape(F * 2, O)

    conn_q = conn.astype(_np_f8)
    maps = []
    for c in range(NCORES):
        cs = conn_q[c * AL : (c + 1) * AL]  # (AL, B=2048, F)
        # pack [mb, bo, bi, f, a]: f-major per bi so stage-1 PSUM columns come
        # out (f, a) and stage-2 rhs slices are contiguous
        cp = np.ascontiguousarray(
            cs.reshape(MB, ABK, BO, BI, F).transpose(0, 2, 3, 4, 1)
        ).reshape(MB * BO, BI, ABK * F)
        bs = bond[c * AL : (c + 1) * AL]  # (AL, F, 2)
        bT = np.ascontiguousarray(bs.transpose(1, 2, 0)).astype(_np_bf16).reshape(
            F * 2, AL
        )
        maps.append(
            {
                "conn": cp,
                "node": node_p,
                "filtT": filtT,
                "bfiltT": bfiltT,
                "bondT": bT,
            }
        )
    return maps


def _enable_tracing():
    """Install the NTFF profile hook (missing antenv.axon_hooks shim) and
    neuter the artifact upload (zero-egress container). Profiling only —
    never touched on the plain kernel() path."""
    import sys
    import types

    try:
        import antenv.axon_hooks  # noqa: F401
    except ImportError:
        from trn_agent_boot.trn_boot import _ntff_profile_via_ctypes

        hook = _ntff_profile_via_ctypes("/opt/axon/libaxon_pjrt.so")
        mod = types.ModuleType("antenv.axon_hooks")
        mod._hook = hook
        mod.get_axon_ntff_profile_hook = lambda: mod._hook
        mod.set_axon_ntff_profile_hook = lambda h: setattr(mod, "_hook", h)
        sys.modules["antenv.axon_hooks"] = mod
        import antenv

        antenv.axon_hooks = mod

    import concourse.bass_utils as _bu

    _bu.upload_artifacts = lambda tmpdir: tmpdir


def run(
    node_property_tensor,
    connectivity_tensor,
    bond_property_tensor,
    filters,
    trace=False,
):
    """Run the sharded kernel; returns (full (A, O) output, exec_time_ns|None)."""
    if trace:
        _enable_tracing()
    nc = _build()
    maps = _in_maps(
        node_property_tensor, connectivity_tensor, bond_property_tensor, filters
    )
    res = run_bass_kernel_spmd(nc, maps, core_ids=list(range(NCORES)), trace=trace)
    parts = [res.results[c]["out"] for c in range(NCORES)]  # each (O, AL)
    full = np.concatenate(parts, axis=1).T  # (A, O)
    return np.ascontiguousarray(full, dtype=np.float32), res.exec_time_ns


def kernel(
    node_property_tensor, connectivity_tensor, bond_property_tensor, filters
) -> np.ndarray:
    out, _ = run(
        node_property_tensor, connectivity_tensor, bond_property_tensor, filters
    )
    return out
